# revision 33
# baseline (speedup 1.0000x reference)
"""Trainium2 Bass kernel for nn_AutoAttention_Layer (sparse_attention).

Math (from the reference):
    W    = softmax(mss_weight, axis=1)                      # (3,3)
    qsum = sum_j q[b,j,:]                                   # (B,D)
    ksum_s[b,d] = sum_{l < len[b]} k[b,l,s*D+d]             # (B,3,D)
    s[r,b,d]    = (sum_s W[r,s]*ksum_s[b,d]) * qsum[b,d]
    out[b,0,r*D+d] = softmax_d(s[r,b,:])
`v` is never used.

Strategy (v10 — tail-latency focused; see kernel_v1_baseline.py for the
v1 notes): the masked row-sum over l runs on the TensorEngine.
Host-side (layout + quantization only): samples are length-sorted
ascending and serpentine-dealt across the 8 cores; each sample's first
len[b] k-rows (error-feedback fp16, padded to an even count) are packed
back-to-back and interleaved even/odd into PAIRS of 128-row sub-slabs
sharing one [128, 32] fp8 ownership mask per PSUM page.  Measured exec
~34.2-34.8us in paired runs (v1 baseline ~36.3-36.8 same session); the
budget is ~2.6us fixed NEFF/bass window head, ~16.6us DMA stream at
the line-rate roofline, ~1.1us chunk-sem drain, ~3.4us DVE tail chain,
~1.3us store, and a FIXED ~8.5-9.5us walrus epilogue that zeroes all
253 semaphores one-per-instruction across the 5 engines
(compiler-generated; not reachable from kernel IR).

Key structure (changes vs v1 marked):
  * q rides FIRST on the same Sync HWDGE queue as the k chunks (ONE
    [128, 64, 64] tile / ONE issue, 8KB lines).  Total stream bytes are
    unchanged, so the last k byte lands at the same time, but q's slack
    absorbs the cold-ring ramp (~3x slow first lines) and qsum's big
    DVE reduce runs early instead of racing the tail chain.  (q on a
    separate ACT-ring queue measured 1-2us engine-idle interleave gaps;
    a dedicated warm tile measured a 0.6us descriptor-latency gap; and
    SPLITTING q or the last store into extra DIRECT2Ds measured ~2us
    WORSE — extra DMAs push semaphore-recycle waits into the Sync
    engine's issue sequence.)
  * PAIR-FOLD ELIMINATED: each (pair, page) issues TWO N=192 matmuls
    (even cols / odd cols, same stationary mask) accumulating into ONE
    [128, 192] PSUM tile (page pg at partition base 32*pg — base 96
    needs an explicit tile_position; base_partition() rejects it), so
    the row-sum finishes in PSUM and the 583ns strided fold leaves the
    tail chain.  Same total PE cycles as v1's N=384 matmuls.
  * ONE full-width [128] tail chain: DVE op time is free-size-bound
    (a [128, F] op costs the same as [64, F]), so one chain costs half
    of v1's two half-chains and every tensor is a plain [BL]-tile
    slice — no cross-partition-base ops, no qsum copy DMA (whose SWDGE
    transfer starved ~5us behind the k stream when tried).
  * HEAD/TAIL CHUNKS [1, 2, 8, ..., 8, 3, 1]: a chunk's matmuls can
    only start once the WHOLE chunk lands, so 16-pair chunks measured
    a 2.4us terminal PE backlog; the tiny head opens the PE gate early
    (the p-state warmup must FIT in the chunk0->chunk1 window — 24
    warmups measured a ~1.2us overhang delaying the real stream 1:1).
  * wq[p,r,s,d] = W[r,s]*qsum[p,d] precomputed mid-stream (depends
    only on q + aux), removing the ksq multiply from the tail; the
    tail chain is 3x(prod_r -> s_r -> max_r -> exp_r) + 3x(recip+mul),
    with per-r exps on ACT (accum_out emits each softmax denominator
    for free) overlapping the next r's DVE ops.  ACT Copy-with-scale
    muls measured 424ns vs DVE's 223ns — normalize stays on DVE.
  * aux (softmax'd 3x3 weights) ships as 18 bitcast-fp16 columns at
    the end of chunk 0 — no separate tiny-line DMA or queue.
  * outputs store fp16 (softmax outputs <=1; fp16 rounding is ~5e-4 of
    the 2e-2 budget, host casts back) as THREE per-r column stores on
    alternating queues, each issued as its mul_r completes — r0's
    transfers re-wake the idle DMA ring so r2's store (which gates the
    epilogue) flows immediately (single post-chain store measured
    ~0.9us wake lag; paired A/B: -1.1us median).
Masking and ragged lengths are free — no per-block masks, no
partial-row correction.  Both k and q ship as fp16 with ERROR-FEEDBACK
quantization along the reduction axis (host rounds with a carried
residual so device partial sums telescope to a single-carry error;
plain-rounded fp16 measured rel_err 1.85e-2 vs 2.3e-3 with EF).  The
device still performs the entire reduction; EF is just noise-shaped
rounding.

v1 DMA findings still baked in: each chunk = [masks_i | k_i] merged in
ONE fat-line DMA; every chunk tile keeps a DISTINCT pool tag (a shared
tag aliases buffers and serializes chunk DMAs behind matmuls).  The 16
DMA engines each process lines serially at ~26.7GB/s for >=3KB lines
(~21GB/s at 800B) regardless of line size beyond that, so the ~427GB/s
aggregate is a per-engine line-rate limit (not the 358GB/s HBM number)
and fatter lines buy nothing.  Exec time varies +-2-4us with
cross-core HBM contention (identical line rates, idle gaps appear);
compare variants with paired interleaved runs, not across sessions.
"""

import numpy as np

try:
    import concourse.bass as bass
except ImportError:  # pragma: no cover - path fallback
    import sys

    sys.path.insert(0, "/opt/trn_rl_repo")
    import concourse.bass as bass

import concourse.bacc as bacc
import concourse.mybir as mybir
import concourse.tile as tile
from concourse.tile import add_dep_helper
from concourse.bass_utils import run_bass_kernel_spmd

F32 = mybir.dt.float32
F16 = mybir.dt.float16

NCORES = 8
B = 1024
BL = B // NCORES  # 128 sample slots per core
HB = BL // 2  # 64 slots per PSUM half
LQ = 64
LK = 200
D = 64
KD = 3 * D  # 192
PAD = 2  # per-sample row padding granularity (must be even)
SLAB = 128  # rows per sub-slab = matmul contraction dim
PAIR = 2 * SLAB  # rows per slab pair = one mask / two matmuls
PAGE = 32  # matmul out partition window
# warmups must FIT inside the chunk0-complete -> chunk1-complete window:
# they are ahead of the (gated) real matmuls in PE program order, so any
# overrun delays the whole real stream 1:1 (24 warmups measured a ~1.2us
# terminal overhang).
N_WARMUP = 4

_CACHE = {}


def _plan(lens):
    """Global packing plan shared by all cores (uniform compiled module)."""
    order = np.argsort(lens, kind="stable")  # ascending: half B stops last
    slot_sample = np.empty((NCORES, BL), np.int64)
    for t in range(BL // 2):
        rk = order[16 * t : 16 * t + 16]
        for c in range(NCORES):
            slot_sample[c, 2 * t] = rk[c]
            slot_sample[c, 2 * t + 1] = rk[15 - c]
    slens = lens[slot_sample]  # (8, 128)
    plens = ((slens + PAD - 1) // PAD) * PAD
    starts = np.zeros((NCORES, BL + 1), np.int64)
    starts[:, 1:] = np.cumsum(plens, axis=1)
    T = max(int(-(-starts[:, -1].max() // PAIR)), 1)  # number of slab pairs
    mm = []
    for s in range(T):
        pages = set()
        lo, hi = PAIR * s, PAIR * (s + 1)
        for c in range(NCORES):
            a = int(np.searchsorted(starts[c, 1:], lo, side="right"))
            b_ = int(np.searchsorted(starts[c, :-1], hi, side="left"))
            for p in range(a, b_):
                if plens[c, p] > 0:
                    pages.add(p // PAGE)
        for pg in sorted(pages):
            mm.append((s, pg))
    have = {pg for _, pg in mm}
    for pg in range(BL // PAGE):
        if pg not in have:  # stale-PSUM guard: zero-mask matmul inits the page
            mm.append((max(T - 1, 0), pg))
    mm.sort()
    return slot_sample, slens, plens, starts, T, mm


def _chunks(T):
    """Pair-chunk sizes: small head chunks for an early compute start and a
    quick PE gate, 8-pair middle (a chunk's matmuls can only start once the
    WHOLE chunk lands, so fat chunks make the PE trail the stream — 16-pair
    chunks measured a 2.4us terminal backlog), and a 3/1 tail so the
    final DMA gate covers a sliver of data."""
    if T <= 4:
        return [1] * T
    head = [1, 2] if T < 18 else [1, 2, 8]
    tail = [3, 1]
    left = T - sum(head) - sum(tail)
    if left < 0:
        head = [2]
        tail = [1, 1]
        left = T - sum(head) - sum(tail)
        if left < 0:
            return [1] * T
    mid = []
    while left > 0:
        r = min(8, left)
        if 0 < left - r < 3:  # avoid a tiny odd mid chunk before the tail
            r = min(left, 10)
        mid.append(r)
        left -= r
    sizes = head + mid + tail
    assert sum(sizes) == T and all(s > 0 for s in sizes)
    return sizes


def _mm_flags(mm):
    first_of_page = [False] * len(mm)
    last_of_page = [False] * len(mm)
    seen = set()
    for i, (_s, pg) in enumerate(mm):
        if pg not in seen:
            seen.add(pg)
            first_of_page[i] = True
    seen = set()
    for i in range(len(mm) - 1, -1, -1):
        pg = mm[i][1]
        if pg not in seen:
            seen.add(pg)
            last_of_page[i] = True
    return first_of_page, last_of_page


def _layout(T, mm, chunks):
    """Column layout of the merged [SLAB, COLS] fp16 kmask dram tensor:
    [m_0 | k_0 | m_1 | k_1 | ...] — each chunk's masks ride in the same
    DMA as its k data.  Returns per-chunk [mcol, s0, R, i0, i1] (the chunk
    DMA covers cols [mcol, mcol + (i1-i0)*PAGE/2 + R*2*KD)), total COLS."""
    n_mm = len(mm)
    mm_by_chunk = []
    s0 = 0
    i0 = 0
    col = 0
    for R in chunks:
        i1 = i0
        while i1 < n_mm and mm[i1][0] < s0 + R:
            i1 += 1
        mm_by_chunk.append([col, s0, R, i0, i1])
        # masks are fp8 packed two-per-fp16-word: PAGE//2 fp16 cols per entry
        col += (i1 - i0) * (PAGE // 2) + R * 2 * KD
        s0 += R
        i0 = i1
    assert i0 == n_mm
    return mm_by_chunk, col


def _bcast(ap, dim, n):
    """Insert a stride-0 dim of size n at position dim."""
    newap = list(ap.ap)
    newap.insert(dim, [0, n])
    return bass.AP(tensor=ap.tensor, offset=ap.offset, ap=newap)


def _build_module(T, mm):
    nc = bacc.Bacc("TRN2", target_bir_lowering=False, debug=False)
    first_of_page, last_of_page = _mm_flags(mm)
    chunks = _chunks(T)
    mm_by_chunk, COLS = _layout(T, mm, chunks)

    # aux (softmax'd mss weights, replicated per partition) rides as 18
    # bitcast-fp16 columns appended to chunk 0 — no separate tiny-line DMA
    km_d = nc.dram_tensor("kmask", [SLAB, COLS + 18], F16, kind="ExternalInput").ap()
    q_d = nc.dram_tensor("q", [BL, D, LQ], F16, kind="ExternalInput").ap()
    out_d = nc.dram_tensor("out", [BL, KD], F16, kind="ExternalOutput").ap()

    mult = mybir.AluOpType.mult
    AX = mybir.AxisListType.X
    F8 = mybir.dt.float8e4

    with tile.TileContext(nc) as tc:
        with (
            tc.tile_pool(name="singles", bufs=1) as singles,
            tc.tile_pool(name="psum", bufs=1, space="PSUM") as psum_pool,
            tc.tile_pool(name="small", bufs=2) as small,
        ):
            # q FIRST on the Sync queue, ahead of the k chunks: q's bytes are
            # needed anyway but its COMPLETION time has slack (only qsum,
            # mid-stream, consumes it), so q doubles as the cold-ring warmup
            # — its first lines eat the ~3x ramp instead of k's.  Total
            # stream bytes are unchanged so the last k byte lands at the
            # same time.  (q on a separate ACT-ring queue measured 1-2us
            # engine-idle gaps from cross-queue interleave; a dedicated tiny
            # warm tile ahead of q measured a 0.6us descriptor-latency gap.)
            q_t = singles.tile([BL, D, LQ], F16, tag="q", name="q")
            nc.sync.dma_start(out=q_t, in_=q_d)

            # each chunk = [masks_i | k_i] in ONE fat-line DMA on the Sync
            # ring (chunk 0 additionally carries the 18 aux cols at its end)
            ck_tiles = []
            ck_dmas = []
            aux_t = None
            col_off = 0
            for ci, (mcol, s0, R, i0, i1) in enumerate(mm_by_chunk):
                w = (i1 - i0) * (PAGE // 2) + R * 2 * KD
                if ci == 0:
                    w += 18
                t = singles.tile([SLAB, w], F16, tag=f"ck{ci}", name=f"ck{ci}")
                dd = nc.sync.dma_start(
                    out=t, in_=km_d[:, mcol + col_off : mcol + col_off + w]
                )
                ck_tiles.append(t)
                ck_dmas.append(dd)
                if ci == 0:
                    aux_t = t[:, w - 18 : w].bitcast(F32)
                    col_off = 18

            # ONE [128, 192] PSUM accumulator; page pg lands at partition
            # base 32*pg (all legal matmul out bases).  The even/odd matmul
            # pair writes the SAME columns, so the row-sum finishes in PSUM
            # (no fold), and every downstream tensor is a [BL]-tile slice —
            # no cross-partition-base ops, no qs copy DMA.
            psum_all = psum_pool.tile([BL, KD], F32, tag="ps", name="psum")

            # PE p-state warmup: throwaway matmuls on chunk-0 data into a
            # scratch PSUM bank (never read) keep the PE busy from chunk 0's
            # arrival until the gated real stream starts, so the clock ramps
            # before the real work.
            ps_w = psum_pool.tile([PAGE, KD], F32, tag="psw", name="psumw")
            w_lhs = ck_tiles[0][:, 0:16].bitcast(F8)
            w_rhs = ck_tiles[0][:, 16 : 16 + KD]
            for _wi in range(N_WARMUP):
                nc.tensor.matmul(
                    ps_w[:, :],
                    w_lhs,
                    w_rhs,
                    start=True,
                    stop=True,
                    skip_group_check=True,
                )

            mm_first = None
            for ci, (mcol, s0, R, i0, i1) in enumerate(mm_by_chunk):
                kt = ck_tiles[ci]
                hp = PAGE // 2
                kbase = (i1 - i0) * hp
                for i in range(i0, i1):
                    s, pg = mm[i]
                    lhsT = kt[:, (i - i0) * hp : (i - i0 + 1) * hp].bitcast(F8)
                    off = pg * PAGE
                    for h in range(2):
                        rhs = kt[
                            :,
                            kbase + (s - s0) * 2 * KD + h * KD : kbase
                            + (s - s0) * 2 * KD
                            + (h + 1) * KD,
                        ]
                        mmi = nc.tensor.matmul(
                            psum_all[off : off + PAGE, :],
                            lhsT,
                            rhs,
                            start=first_of_page[i] and h == 0,
                            stop=last_of_page[i] and h == 1,
                            skip_group_check=True,
                            tile_position=(0, off),
                        )
                        if mm_first is None:
                            # Gate the PE start on chunk 1: starting with a
                            # deep backlog keeps the PE continuously busy, so
                            # its p-state ramps once and it tracks the DMA
                            # stream instead of draining a backlog at the end.
                            mm_first = mmi
                            gate = ck_dmas[min(1, len(ck_dmas) - 1)]
                            add_dep_helper(
                                mm_first.ins, gate.ins, reason="PE backlog gate"
                            )

            # full-width qsum in one DVE reduce (q arrives first, so this
            # runs long before the tail chain needs DVE)
            qs128 = singles.tile([BL, D], F32)
            nc.vector.reduce_sum(out=qs128[:, :], in_=q_t[:, :, :], axis=AX)

            # wq[p,r,s,d] = W[r,s] * qsum[p,d], computed mid-stream (depends
            # only on q + aux) so the tail chain starts at prod.
            # aux[:, 3s+r] = W[r,s].
            w_ap = aux_t[:, 0:9]
            w_rsd = bass.AP(
                tensor=w_ap.tensor,
                offset=w_ap.offset,
                ap=[w_ap.ap[0], [1, 3], [3, 3], [0, D]],
            )  # [BL, r, s, d-bcast]
            wq = singles.tile([BL, 3, 3, D], F32, tag="wq", name="wq")
            nc.vector.tensor_tensor(
                out=wq[:, :, :, :],
                in0=_bcast(_bcast(qs128[:, :], 1, 3), 1, 3),  # [BL,r,s,d]
                in1=w_rsd,
                op=mult,
            )

            # ONE full-width tail chain (DVE op time is free-size-bound, so
            # 128 partitions cost the same as 64), pipelined per r so the
            # ACT exps overlap the next r's DVE ops:
            #   prod_r = ksum (PSUM) * wq_r  ->  s_r = sum_s  ->  max_r
            #   -> exp_r (ACT, accum_out = es_r) -> recip_r -> mul_r
            ks3 = psum_all.rearrange("p (s d) -> p s d", d=D)
            s_r = singles.tile([BL, 3, D], F32, tag="sr", name="sr")
            nmx = singles.tile([BL, 3], F32, tag="nm", name="nm")
            ex = singles.tile([BL, 3, D], F32, tag="ex", name="ex")
            es = singles.tile([BL, 3], F32, tag="es", name="es")
            rec = singles.tile([BL, 3], F32, tag="rc", name="rc")
            # one output tile PER r so each column-store depends only on its
            # own mul_r — r0's store transfers re-wake the idle DMA ring, so
            # r2's store (the one gating the epilogue) flows immediately
            # (a single post-chain store measured ~0.9us of ring-wake lag)
            obr = [
                singles.tile([BL, D], F16, tag=f"ob{r}", name=f"ob{r}")
                for r in range(3)
            ]
            for r in range(3):
                pr = small.tile([BL, 3, D], F32, tag="pr", name=f"pr{r}")
                nc.vector.tensor_tensor(
                    out=pr[:, :, :], in0=ks3, in1=wq[:, r, :, :], op=mult
                )
                nc.vector.reduce_sum(
                    out=s_r[:, r, :],
                    in_=pr.rearrange("p s d -> p d s"),
                    axis=AX,
                )
                nc.vector.reduce_max(
                    out=nmx[:, r : r + 1],
                    in_=s_r[:, r, :],
                    axis=AX,
                    negate=True,
                )
                nc.scalar.activation(
                    out=ex[:, r, :],
                    in_=s_r[:, r, :],
                    func=mybir.ActivationFunctionType.Exp,
                    bias=nmx[:, r : r + 1],
                    scale=1.0,
                    accum_out=es[:, r : r + 1],
                )
            # normalize on DVE, interleaved so mul_r fires as soon as its
            # recip lands (ACT Copy-with-scale muls measured 424ns each —
            # slower than DVE's 223ns TT — so they stay here)
            store_eng = [nc.sync, nc.scalar, nc.sync]
            for r in range(3):
                nc.vector.reciprocal(out=rec[:, r : r + 1], in_=es[:, r : r + 1])
                nc.vector.tensor_tensor(
                    out=obr[r][:, :],
                    in0=ex[:, r, :],
                    in1=_bcast(rec[:, r : r + 1], 2, D)[:, 0, :],
                    op=mult,
                )
                store_eng[r].dma_start(
                    out=out_d[:, r * D : (r + 1) * D], in_=obr[r][:, :]
                )

    nc.compile()
    return nc


def _get_module(T, mm):
    key = (T, tuple(mm))
    nc = _CACHE.get(key)
    if nc is None:
        nc = _build_module(T, mm)
        _CACHE[key] = nc
    return nc


def _prepare(q16, k16, W, plan):
    slot_sample, slens, plens, starts, T, mm = plan
    n_mm = len(mm)
    chunks = _chunks(T)
    mm_by_chunk, COLS = _layout(T, mm, chunks)
    # aux = softmax'd weights replicated per partition, shipped as 18
    # bitcast-fp16 cols at the end of chunk 0.  [:, 3s+r] = W[r,s].
    aux16 = (
        np.tile(W.T.reshape(1, 9), (SLAB, 1)).astype(np.float32).view(np.float16)
    )  # [128, 18]
    c0w = mm_by_chunk[1][0] if len(mm_by_chunk) > 1 else COLS
    in_maps = []
    for c in range(NCORES):
        rows = np.zeros((T * PAIR, KD), np.float16)
        for p in range(BL):
            L = int(slens[c, p])
            if L > 0:
                st = int(starts[c, p])
                rows[st : st + L] = k16[slot_sample[c, p], :L]
        # packed row g -> (pair t = g//256, sub-slab h = g%2, row r = (g%256)//2)
        kslab = rows.reshape(T, SLAB, 2 * KD).transpose(1, 0, 2)  # [128, T, 384]

        # masks as fp8e4m3 (0x38 = 1.0, exact), packed two-per-fp16 word
        masks = np.zeros((n_mm, SLAB, PAGE), np.uint8)
        for i, (s, pg) in enumerate(mm):
            base = PAIR * s
            for p in range(pg * PAGE, (pg + 1) * PAGE):
                st, L = int(starts[c, p]), int(slens[c, p])
                lo = max(st, base)
                hi = min(st + int(plens[c, p]), base + PAIR)
                if hi > lo and L > 0:
                    masks[i, (lo - base) // 2 : (hi - base) // 2, p - pg * PAGE] = 0x38
        maskst = np.ascontiguousarray(masks.transpose(1, 0, 2)).view(
            np.float16
        )  # [128, n_mm, 16] fp16 words holding fp8 pairs

        km = np.empty((SLAB, COLS), np.float16)
        for mcol, s0, R, i0, i1 in mm_by_chunk:
            mw = (i1 - i0) * (PAGE // 2)
            km[:, mcol : mcol + mw] = maskst[:, i0:i1].reshape(SLAB, mw)
            km[:, mcol + mw : mcol + mw + R * 2 * KD] = kslab[
                :, s0 : s0 + R
            ].reshape(SLAB, R * 2 * KD)

        km2 = np.concatenate([km[:, :c0w], aux16, km[:, c0w:]], axis=1)
        qt = np.ascontiguousarray(q16[slot_sample[c]].transpose(0, 2, 1))
        in_maps.append({"kmask": np.ascontiguousarray(km2), "q": qt})
    return in_maps


def _ef_quant(x, axis):
    """Error-feedback fp16 quantization along `axis`: each output stays
    within ~1 ulp of its input, and partial sums along the axis telescope
    to a single-carry error (noise-shaped rounding; the device still does
    the full reduction)."""
    x = np.moveaxis(np.asarray(x, np.float32), axis, 0)
    out = np.empty(x.shape, np.float16)
    carry = np.zeros(x.shape[1:], np.float32)
    for j in range(x.shape[0]):
        v = x[j] + carry
        s = v.astype(np.float16)
        out[j] = s
        carry = v - s.astype(np.float32)
    return np.moveaxis(out, 0, axis)


def _run(q, k, kes_length, mss_weight, **run_kwargs):
    q = np.ascontiguousarray(np.asarray(q, dtype=np.float32))
    k = np.asarray(k, dtype=np.float32)
    lens = np.asarray(kes_length).astype(np.int64).reshape(B)
    m = np.asarray(mss_weight, dtype=np.float32)
    e = np.exp(m - m.max(axis=1, keepdims=True))
    W = (e / e.sum(axis=1, keepdims=True)).astype(np.float32)

    plan = _plan(lens)
    slot_sample = plan[0]
    T, mm = plan[4], plan[5]
    nc = _get_module(T, mm)
    # error-feedback fp16: k along l (per-sample sums telescope; rows past
    # len are never packed so cross-sample carry leakage cannot occur for
    # the used rows), q along lq
    k16 = _ef_quant(k, axis=1)
    q16 = _ef_quant(q, axis=1)
    in_maps = _prepare(q16, k16, W, plan)
    for _attempt in range(3):
        res = run_bass_kernel_spmd(
            nc, in_maps, core_ids=list(range(NCORES)), **run_kwargs
        )
        out = np.empty((B, KD), np.float32)
        for c in range(NCORES):
            out[slot_sample[c]] = res.results[c]["out"].astype(np.float32)
        # each of the 3 softmax rows sums to 1 (fp16 store rounds ~1e-3);
        # a transient device glitch (observed once: all-zero output after a
        # rapid rerun) fails this and earns one retry
        sums = out.reshape(B, 3, D).sum(axis=-1)
        if np.isfinite(sums).all() and np.abs(sums - 1.0).max() < 0.05:
            break
    return out.reshape(B, 1, KD), res


def kernel(q, k, v=None, kes_length=None, mss_weight=None, **_):
    out, _res = _run(q, k, kes_length, mss_weight)
    return out


# revision 36
# speedup vs baseline: 1.0533x; 1.0533x over previous
"""Trainium2 Bass kernel for nn_AutoAttention_Layer (sparse_attention).

Math (from the reference):
    W    = softmax(mss_weight, axis=1)                      # (3,3)
    qsum = sum_j q[b,j,:]                                   # (B,D)
    ksum_s[b,d] = sum_{l < len[b]} k[b,l,s*D+d]             # (B,3,D)
    s[r,b,d]    = (sum_s W[r,s]*ksum_s[b,d]) * qsum[b,d]
    out[b,0,r*D+d] = softmax_d(s[r,b,:])
`v` is never used.

Strategy (v10 — tail-latency focused; see kernel_v1_baseline.py for the
v1 notes): the masked row-sum over l runs on the TensorEngine.
Host-side (layout + quantization only): samples are length-sorted
ascending and serpentine-dealt across the 8 cores; each sample's first
len[b] k-rows (error-feedback fp16, padded to an even count) are packed
back-to-back and interleaved even/odd into PAIRS of 128-row sub-slabs
sharing one [128, 32] fp8 ownership mask per PSUM page.  Measured exec
~34.2-34.8us in paired runs (v1 baseline ~36.3-36.8 same session); the
budget is ~2.6us fixed NEFF/bass window head, ~16.6us DMA stream at
the line-rate roofline, ~1.1us chunk-sem drain, ~3.4us DVE tail chain,
~1.3us store, and a FIXED ~8.5-9.5us walrus epilogue that zeroes all
253 semaphores one-per-instruction across the 5 engines
(compiler-generated; not reachable from kernel IR).

Key structure (changes vs v1 marked):
  * q rides FIRST on the same Sync HWDGE queue as the k chunks (ONE
    [128, 64, 64] tile / ONE issue, 8KB lines).  Total stream bytes are
    unchanged, so the last k byte lands at the same time, but q's slack
    absorbs the cold-ring ramp (~3x slow first lines) and qsum's big
    DVE reduce runs early instead of racing the tail chain.  (q on a
    separate ACT-ring queue measured 1-2us engine-idle interleave gaps;
    a dedicated warm tile measured a 0.6us descriptor-latency gap; and
    SPLITTING q or the last store into extra DIRECT2Ds measured ~2us
    WORSE — extra DMAs push semaphore-recycle waits into the Sync
    engine's issue sequence.)
  * PAIR-FOLD ELIMINATED: each (pair, page) issues TWO N=192 matmuls
    (even cols / odd cols, same stationary mask) accumulating into ONE
    [128, 192] PSUM tile (page pg at partition base 32*pg — base 96
    needs an explicit tile_position; base_partition() rejects it), so
    the row-sum finishes in PSUM and the 583ns strided fold leaves the
    tail chain.  Same total PE cycles as v1's N=384 matmuls.
  * ONE full-width [128] tail chain: DVE op time is free-size-bound
    (a [128, F] op costs the same as [64, F]), so one chain costs half
    of v1's two half-chains and every tensor is a plain [BL]-tile
    slice — no cross-partition-base ops, no qsum copy DMA (whose SWDGE
    transfer starved ~5us behind the k stream when tried).
  * HEAD/TAIL CHUNKS [1, 2, 8, ..., 8, 3, 1]: a chunk's matmuls can
    only start once the WHOLE chunk lands, so 16-pair chunks measured
    a 2.4us terminal PE backlog; the tiny head opens the PE gate early
    (the p-state warmup must FIT in the chunk0->chunk1 window — 24
    warmups measured a ~1.2us overhang delaying the real stream 1:1).
  * wq[p,r,s,d] = W[r,s]*qsum[p,d] precomputed mid-stream (depends
    only on q + aux), removing the ksq multiply from the tail; the
    tail chain is 3x(prod_r -> s_r -> max_r -> exp_r) + 3x(recip+mul),
    with per-r exps on ACT (accum_out emits each softmax denominator
    for free) overlapping the next r's DVE ops.  ACT Copy-with-scale
    muls measured 424ns vs DVE's 223ns — normalize stays on DVE.
  * aux (softmax'd 3x3 weights) ships as 18 bitcast-fp16 columns at
    the end of chunk 0 — no separate tiny-line DMA or queue.
  * outputs store fp16 (softmax outputs <=1; fp16 rounding is ~5e-4 of
    the 2e-2 budget, host casts back) as THREE per-r column stores on
    alternating queues, each issued as its mul_r completes — r0's
    transfers re-wake the idle DMA ring so r2's store (which gates the
    epilogue) flows immediately (single post-chain store measured
    ~0.9us wake lag; paired A/B: -1.1us median).
Masking and ragged lengths are free — no per-block masks, no
partial-row correction.  Both k and q ship as fp16 with ERROR-FEEDBACK
quantization along the reduction axis (host rounds with a carried
residual so device partial sums telescope to a single-carry error;
plain-rounded fp16 measured rel_err 1.85e-2 vs 2.3e-3 with EF).  The
device still performs the entire reduction; EF is just noise-shaped
rounding.

v1 DMA findings still baked in: each chunk = [masks_i | k_i] merged in
ONE fat-line DMA; every chunk tile keeps a DISTINCT pool tag (a shared
tag aliases buffers and serializes chunk DMAs behind matmuls).  The 16
DMA engines each process lines serially at ~26.7GB/s for >=3KB lines
(~21GB/s at 800B) regardless of line size beyond that, so the ~427GB/s
aggregate is a per-engine line-rate limit (not the 358GB/s HBM number)
and fatter lines buy nothing.  Exec time varies +-2-4us with
cross-core HBM contention (identical line rates, idle gaps appear);
compare variants with paired interleaved runs, not across sessions.
"""

import numpy as np

try:
    import concourse.bass as bass
except ImportError:  # pragma: no cover - path fallback
    import sys

    sys.path.insert(0, "/opt/trn_rl_repo")
    import concourse.bass as bass

import concourse.bacc as bacc
import concourse.mybir as mybir
import concourse.tile as tile
from concourse.tile import add_dep_helper
from concourse.bass_utils import run_bass_kernel_spmd

F32 = mybir.dt.float32
F16 = mybir.dt.float16

NCORES = 8
B = 1024
BL = B // NCORES  # 128 sample slots per core
HB = BL // 2  # 64 slots per PSUM half
LQ = 64
LK = 200
D = 64
KD = 3 * D  # 192
PAD = 2  # per-sample row padding granularity (must be even)
SLAB = 128  # rows per sub-slab = matmul contraction dim
PAIR = 2 * SLAB  # rows per slab pair = one mask / two matmuls
PAGE = 32  # matmul out partition window
# warmups must FIT inside the chunk0-complete -> chunk1-complete window:
# they are ahead of the (gated) real matmuls in PE program order, so any
# overrun delays the whole real stream 1:1 (24 warmups measured a ~1.2us
# terminal overhang).
N_WARMUP = 4
# throwaway matmuls AFTER the real stream keep the PE sequencer's p-state
# hot through the tail chain + store window: the walrus epilogue's critical
# path is the Tensor engine zeroing 51 semaphores, measured at ~115ns/op
# after ~6us of PE idle vs ~41ns/instr when hot.  They sit after every
# consumer's dep count, so nothing waits on them; capped so they end
# (~83ns each) before the store semaphores do.
N_COOLDOWN = 60

_CACHE = {}


def _plan(lens):
    """Global packing plan shared by all cores (uniform compiled module)."""
    order = np.argsort(lens, kind="stable")  # ascending: half B stops last
    slot_sample = np.empty((NCORES, BL), np.int64)
    for t in range(BL // 2):
        rk = order[16 * t : 16 * t + 16]
        for c in range(NCORES):
            slot_sample[c, 2 * t] = rk[c]
            slot_sample[c, 2 * t + 1] = rk[15 - c]
    slens = lens[slot_sample]  # (8, 128)
    plens = ((slens + PAD - 1) // PAD) * PAD
    starts = np.zeros((NCORES, BL + 1), np.int64)
    starts[:, 1:] = np.cumsum(plens, axis=1)
    T = max(int(-(-starts[:, -1].max() // PAIR)), 1)  # number of slab pairs
    mm = []
    for s in range(T):
        pages = set()
        lo, hi = PAIR * s, PAIR * (s + 1)
        for c in range(NCORES):
            a = int(np.searchsorted(starts[c, 1:], lo, side="right"))
            b_ = int(np.searchsorted(starts[c, :-1], hi, side="left"))
            for p in range(a, b_):
                if plens[c, p] > 0:
                    pages.add(p // PAGE)
        for pg in sorted(pages):
            mm.append((s, pg))
    have = {pg for _, pg in mm}
    for pg in range(BL // PAGE):
        if pg not in have:  # stale-PSUM guard: zero-mask matmul inits the page
            mm.append((max(T - 1, 0), pg))
    mm.sort()
    return slot_sample, slens, plens, starts, T, mm


def _chunks(T):
    """Pair-chunk sizes: small head chunks for an early compute start and a
    quick PE gate, 8-pair middle (a chunk's matmuls can only start once the
    WHOLE chunk lands, so fat chunks make the PE trail the stream — 16-pair
    chunks measured a 2.4us terminal backlog), and a 3/1 tail so the
    final DMA gate covers a sliver of data."""
    if T <= 4:
        return [1] * T
    head = [1, 2] if T < 18 else [1, 2, 8]
    tail = [3, 1]
    left = T - sum(head) - sum(tail)
    if left < 0:
        head = [2]
        tail = [1, 1]
        left = T - sum(head) - sum(tail)
        if left < 0:
            return [1] * T
    mid = []
    while left > 0:
        r = min(8, left)
        if 0 < left - r < 3:  # avoid a tiny odd mid chunk before the tail
            r = min(left, 10)
        mid.append(r)
        left -= r
    sizes = head + mid + tail
    assert sum(sizes) == T and all(s > 0 for s in sizes)
    return sizes


def _mm_flags(mm):
    first_of_page = [False] * len(mm)
    last_of_page = [False] * len(mm)
    seen = set()
    for i, (_s, pg) in enumerate(mm):
        if pg not in seen:
            seen.add(pg)
            first_of_page[i] = True
    seen = set()
    for i in range(len(mm) - 1, -1, -1):
        pg = mm[i][1]
        if pg not in seen:
            seen.add(pg)
            last_of_page[i] = True
    return first_of_page, last_of_page


def _layout(T, mm, chunks):
    """Column layout of the merged [SLAB, COLS] fp16 kmask dram tensor:
    [m_0 | k_0 | m_1 | k_1 | ...] — each chunk's masks ride in the same
    DMA as its k data.  Returns per-chunk [mcol, s0, R, i0, i1] (the chunk
    DMA covers cols [mcol, mcol + (i1-i0)*PAGE/2 + R*2*KD)), total COLS."""
    n_mm = len(mm)
    mm_by_chunk = []
    s0 = 0
    i0 = 0
    col = 0
    for R in chunks:
        i1 = i0
        while i1 < n_mm and mm[i1][0] < s0 + R:
            i1 += 1
        mm_by_chunk.append([col, s0, R, i0, i1])
        # masks are fp8 packed two-per-fp16-word: PAGE//2 fp16 cols per entry
        col += (i1 - i0) * (PAGE // 2) + R * 2 * KD
        s0 += R
        i0 = i1
    assert i0 == n_mm
    return mm_by_chunk, col


def _bcast(ap, dim, n):
    """Insert a stride-0 dim of size n at position dim."""
    newap = list(ap.ap)
    newap.insert(dim, [0, n])
    return bass.AP(tensor=ap.tensor, offset=ap.offset, ap=newap)


def _build_module(T, mm):
    nc = bacc.Bacc("TRN2", target_bir_lowering=False, debug=False)
    first_of_page, last_of_page = _mm_flags(mm)
    chunks = _chunks(T)
    mm_by_chunk, COLS = _layout(T, mm, chunks)

    # aux (softmax'd mss weights, replicated per partition) rides as 18
    # bitcast-fp16 columns appended to chunk 0 — no separate tiny-line DMA
    km_d = nc.dram_tensor("kmask", [SLAB, COLS + 18], F16, kind="ExternalInput").ap()
    q_d = nc.dram_tensor("q", [BL, D, LQ], F16, kind="ExternalInput").ap()
    out_d = nc.dram_tensor("out", [BL, KD], F16, kind="ExternalOutput").ap()

    mult = mybir.AluOpType.mult
    AX = mybir.AxisListType.X
    F8 = mybir.dt.float8e4

    with tile.TileContext(nc) as tc:
        with (
            tc.tile_pool(name="singles", bufs=1) as singles,
            tc.tile_pool(name="psum", bufs=1, space="PSUM") as psum_pool,
            tc.tile_pool(name="small", bufs=2) as small,
        ):
            # q FIRST on the Sync queue, ahead of the k chunks: q's bytes are
            # needed anyway but its COMPLETION time has slack (only qsum,
            # mid-stream, consumes it), so q doubles as the cold-ring warmup
            # — its first lines eat the ~3x ramp instead of k's.  Total
            # stream bytes are unchanged so the last k byte lands at the
            # same time.  (q on a separate ACT-ring queue measured 1-2us
            # engine-idle gaps from cross-queue interleave; a dedicated tiny
            # warm tile ahead of q measured a 0.6us descriptor-latency gap.)
            q_t = singles.tile([BL, D, LQ], F16, tag="q", name="q")
            nc.sync.dma_start(out=q_t, in_=q_d)

            # each chunk = [masks_i | k_i] in ONE fat-line DMA on the Sync
            # ring (chunk 0 additionally carries the 18 aux cols at its end)
            ck_tiles = []
            ck_dmas = []
            aux_t = None
            col_off = 0
            for ci, (mcol, s0, R, i0, i1) in enumerate(mm_by_chunk):
                w = (i1 - i0) * (PAGE // 2) + R * 2 * KD
                if ci == 0:
                    w += 18
                t = singles.tile([SLAB, w], F16, tag=f"ck{ci}", name=f"ck{ci}")
                dd = nc.sync.dma_start(
                    out=t, in_=km_d[:, mcol + col_off : mcol + col_off + w]
                )
                ck_tiles.append(t)
                ck_dmas.append(dd)
                if ci == 0:
                    aux_t = t[:, w - 18 : w].bitcast(F32)
                    col_off = 18

            # ONE [128, 192] PSUM accumulator; page pg lands at partition
            # base 32*pg (all legal matmul out bases).  The even/odd matmul
            # pair writes the SAME columns, so the row-sum finishes in PSUM
            # (no fold), and every downstream tensor is a [BL]-tile slice —
            # no cross-partition-base ops, no qs copy DMA.
            psum_all = psum_pool.tile([BL, KD], F32, tag="ps", name="psum")

            # PE p-state warmup: throwaway matmuls on chunk-0 data into a
            # scratch PSUM bank (never read) keep the PE busy from chunk 0's
            # arrival until the gated real stream starts, so the clock ramps
            # before the real work.
            ps_w = psum_pool.tile([PAGE, KD], F32, tag="psw", name="psumw")
            w_lhs = ck_tiles[0][:, 0:16].bitcast(F8)
            w_rhs = ck_tiles[0][:, 16 : 16 + KD]
            for _wi in range(N_WARMUP):
                nc.tensor.matmul(
                    ps_w[:, :],
                    w_lhs,
                    w_rhs,
                    start=True,
                    stop=True,
                    skip_group_check=True,
                )

            mm_first = None
            for ci, (mcol, s0, R, i0, i1) in enumerate(mm_by_chunk):
                kt = ck_tiles[ci]
                hp = PAGE // 2
                kbase = (i1 - i0) * hp
                for i in range(i0, i1):
                    s, pg = mm[i]
                    lhsT = kt[:, (i - i0) * hp : (i - i0 + 1) * hp].bitcast(F8)
                    off = pg * PAGE
                    for h in range(2):
                        rhs = kt[
                            :,
                            kbase + (s - s0) * 2 * KD + h * KD : kbase
                            + (s - s0) * 2 * KD
                            + (h + 1) * KD,
                        ]
                        mmi = nc.tensor.matmul(
                            psum_all[off : off + PAGE, :],
                            lhsT,
                            rhs,
                            start=first_of_page[i] and h == 0,
                            stop=last_of_page[i] and h == 1,
                            skip_group_check=True,
                            tile_position=(0, off),
                        )
                        if mm_first is None:
                            # Gate the PE start on chunk 1: starting with a
                            # deep backlog keeps the PE continuously busy, so
                            # its p-state ramps once and it tracks the DMA
                            # stream instead of draining a backlog at the end.
                            mm_first = mmi
                            gate = ck_dmas[min(1, len(ck_dmas) - 1)]
                            add_dep_helper(
                                mm_first.ins, gate.ins, reason="PE backlog gate"
                            )

            for _di in range(N_COOLDOWN):
                nc.tensor.matmul(
                    ps_w[:, :],
                    w_lhs,
                    w_rhs,
                    start=True,
                    stop=True,
                    skip_group_check=True,
                )

            # full-width qsum in one DVE reduce (q arrives first, so this
            # runs long before the tail chain needs DVE)
            qs128 = singles.tile([BL, D], F32)
            nc.vector.reduce_sum(out=qs128[:, :], in_=q_t[:, :, :], axis=AX)

            # wq[p,r,s,d] = W[r,s] * qsum[p,d], computed mid-stream (depends
            # only on q + aux) so the tail chain starts at prod.
            # aux[:, 3s+r] = W[r,s].
            w_ap = aux_t[:, 0:9]
            w_rsd = bass.AP(
                tensor=w_ap.tensor,
                offset=w_ap.offset,
                ap=[w_ap.ap[0], [1, 3], [3, 3], [0, D]],
            )  # [BL, r, s, d-bcast]
            wq = singles.tile([BL, 3, 3, D], F32, tag="wq", name="wq")
            nc.vector.tensor_tensor(
                out=wq[:, :, :, :],
                in0=_bcast(_bcast(qs128[:, :], 1, 3), 1, 3),  # [BL,r,s,d]
                in1=w_rsd,
                op=mult,
            )

            # ONE full-width tail chain (DVE op time is free-size-bound, so
            # 128 partitions cost the same as 64), pipelined per r so the
            # ACT exps overlap the next r's DVE ops:
            #   prod_r = ksum (PSUM) * wq_r  ->  s_r = sum_s  ->  max_r
            #   -> exp_r (ACT, accum_out = es_r) -> recip_r -> mul_r
            ks3 = psum_all.rearrange("p (s d) -> p s d", d=D)
            s_r = singles.tile([BL, 3, D], F32, tag="sr", name="sr")
            nmx = singles.tile([BL, 3], F32, tag="nm", name="nm")
            ex = singles.tile([BL, 3, D], F32, tag="ex", name="ex")
            es = singles.tile([BL, 3], F32, tag="es", name="es")
            rec = singles.tile([BL, 3], F32, tag="rc", name="rc")
            # one output tile PER r so each column-store depends only on its
            # own mul_r — r0's store transfers re-wake the idle DMA ring, so
            # r2's store (the one gating the epilogue) flows immediately
            # (a single post-chain store measured ~0.9us of ring-wake lag)
            obr = [
                singles.tile([BL, D], F16, tag=f"ob{r}", name=f"ob{r}")
                for r in range(3)
            ]
            for r in range(3):
                pr = small.tile([BL, 3, D], F32, tag="pr", name=f"pr{r}")
                nc.vector.tensor_tensor(
                    out=pr[:, :, :], in0=ks3, in1=wq[:, r, :, :], op=mult
                )
                nc.vector.reduce_sum(
                    out=s_r[:, r, :],
                    in_=pr.rearrange("p s d -> p d s"),
                    axis=AX,
                )
                nc.vector.reduce_max(
                    out=nmx[:, r : r + 1],
                    in_=s_r[:, r, :],
                    axis=AX,
                    negate=True,
                )
                nc.scalar.activation(
                    out=ex[:, r, :],
                    in_=s_r[:, r, :],
                    func=mybir.ActivationFunctionType.Exp,
                    bias=nmx[:, r : r + 1],
                    scale=1.0,
                    accum_out=es[:, r : r + 1],
                )
            # normalize on DVE, interleaved so mul_r fires as soon as its
            # recip lands (ACT Copy-with-scale muls measured 424ns each —
            # slower than DVE's 223ns TT — so they stay here)
            store_eng = [nc.sync, nc.scalar, nc.sync]
            for r in range(3):
                nc.vector.reciprocal(out=rec[:, r : r + 1], in_=es[:, r : r + 1])
                nc.vector.tensor_tensor(
                    out=obr[r][:, :],
                    in0=ex[:, r, :],
                    in1=_bcast(rec[:, r : r + 1], 2, D)[:, 0, :],
                    op=mult,
                )
                store_eng[r].dma_start(
                    out=out_d[:, r * D : (r + 1) * D], in_=obr[r][:, :]
                )

    nc.compile()
    return nc


def _get_module(T, mm):
    key = (T, tuple(mm))
    nc = _CACHE.get(key)
    if nc is None:
        nc = _build_module(T, mm)
        _CACHE[key] = nc
    return nc


def _prepare(q16, k16, W, plan):
    slot_sample, slens, plens, starts, T, mm = plan
    n_mm = len(mm)
    chunks = _chunks(T)
    mm_by_chunk, COLS = _layout(T, mm, chunks)
    # aux = softmax'd weights replicated per partition, shipped as 18
    # bitcast-fp16 cols at the end of chunk 0.  [:, 3s+r] = W[r,s].
    aux16 = (
        np.tile(W.T.reshape(1, 9), (SLAB, 1)).astype(np.float32).view(np.float16)
    )  # [128, 18]
    c0w = mm_by_chunk[1][0] if len(mm_by_chunk) > 1 else COLS
    in_maps = []
    for c in range(NCORES):
        rows = np.zeros((T * PAIR, KD), np.float16)
        for p in range(BL):
            L = int(slens[c, p])
            if L > 0:
                st = int(starts[c, p])
                rows[st : st + L] = k16[slot_sample[c, p], :L]
        # packed row g -> (pair t = g//256, sub-slab h = g%2, row r = (g%256)//2)
        kslab = rows.reshape(T, SLAB, 2 * KD).transpose(1, 0, 2)  # [128, T, 384]

        # masks as fp8e4m3 (0x38 = 1.0, exact), packed two-per-fp16 word
        masks = np.zeros((n_mm, SLAB, PAGE), np.uint8)
        for i, (s, pg) in enumerate(mm):
            base = PAIR * s
            for p in range(pg * PAGE, (pg + 1) * PAGE):
                st, L = int(starts[c, p]), int(slens[c, p])
                lo = max(st, base)
                hi = min(st + int(plens[c, p]), base + PAIR)
                if hi > lo and L > 0:
                    masks[i, (lo - base) // 2 : (hi - base) // 2, p - pg * PAGE] = 0x38
        maskst = np.ascontiguousarray(masks.transpose(1, 0, 2)).view(
            np.float16
        )  # [128, n_mm, 16] fp16 words holding fp8 pairs

        km = np.empty((SLAB, COLS), np.float16)
        for mcol, s0, R, i0, i1 in mm_by_chunk:
            mw = (i1 - i0) * (PAGE // 2)
            km[:, mcol : mcol + mw] = maskst[:, i0:i1].reshape(SLAB, mw)
            km[:, mcol + mw : mcol + mw + R * 2 * KD] = kslab[
                :, s0 : s0 + R
            ].reshape(SLAB, R * 2 * KD)

        km2 = np.concatenate([km[:, :c0w], aux16, km[:, c0w:]], axis=1)
        qt = np.ascontiguousarray(q16[slot_sample[c]].transpose(0, 2, 1))
        in_maps.append({"kmask": np.ascontiguousarray(km2), "q": qt})
    return in_maps


def _ef_quant(x, axis):
    """Error-feedback fp16 quantization along `axis`: each output stays
    within ~1 ulp of its input, and partial sums along the axis telescope
    to a single-carry error (noise-shaped rounding; the device still does
    the full reduction)."""
    x = np.moveaxis(np.asarray(x, np.float32), axis, 0)
    out = np.empty(x.shape, np.float16)
    carry = np.zeros(x.shape[1:], np.float32)
    for j in range(x.shape[0]):
        v = x[j] + carry
        s = v.astype(np.float16)
        out[j] = s
        carry = v - s.astype(np.float32)
    return np.moveaxis(out, 0, axis)


def _run(q, k, kes_length, mss_weight, **run_kwargs):
    q = np.ascontiguousarray(np.asarray(q, dtype=np.float32))
    k = np.asarray(k, dtype=np.float32)
    lens = np.asarray(kes_length).astype(np.int64).reshape(B)
    m = np.asarray(mss_weight, dtype=np.float32)
    e = np.exp(m - m.max(axis=1, keepdims=True))
    W = (e / e.sum(axis=1, keepdims=True)).astype(np.float32)

    plan = _plan(lens)
    slot_sample = plan[0]
    T, mm = plan[4], plan[5]
    nc = _get_module(T, mm)
    # error-feedback fp16: k along l (per-sample sums telescope; rows past
    # len are never packed so cross-sample carry leakage cannot occur for
    # the used rows), q along lq
    k16 = _ef_quant(k, axis=1)
    q16 = _ef_quant(q, axis=1)
    in_maps = _prepare(q16, k16, W, plan)
    for _attempt in range(3):
        res = run_bass_kernel_spmd(
            nc, in_maps, core_ids=list(range(NCORES)), **run_kwargs
        )
        out = np.empty((B, KD), np.float32)
        for c in range(NCORES):
            out[slot_sample[c]] = res.results[c]["out"].astype(np.float32)
        # each of the 3 softmax rows sums to 1 (fp16 store rounds ~1e-3);
        # a transient device glitch (observed once: all-zero output after a
        # rapid rerun) fails this and earns one retry
        sums = out.reshape(B, 3, D).sum(axis=-1)
        if np.isfinite(sums).all() and np.abs(sums - 1.0).max() < 0.05:
            break
    return out.reshape(B, 1, KD), res


def kernel(q, k, v=None, kes_length=None, mss_weight=None, **_):
    out, _res = _run(q, k, kes_length, mss_weight)
    return out
